# revision 40
# baseline (speedup 1.0000x reference)
"""MoE routing layer on 8 Trainium2 NeuronCores (data-parallel over batch).

Per core (4 samples):
  routing MLP -> exp(cosine sim vs embeddings) -> unnormalized weights
  e[4,10]; w_eff[b] = sum_n e[b,n] * conv_w[n] (conv linear in weights);
  conv = 9 shifted bf16 matmuls over the flat 58-wide grid, all 4
  samples concurrent via 4-quadrant PE tiling; softmax normalization
  (1/sum e) and conv bias fold into the PSUM->SBUF staging op.

w_eff is built two ways concurrently: col-half 0 on the PE as a chain
of diagonal-stationary matmuls accumulating in PSUM (diag(wfbc_n) @
cwp_n), col-half 1 on DVE as tensor_scalar products + tensor_tensor
adds (ACT helps with late products). All heavy data is bf16; PSUM is
fp32. Host pre-packs: x in 4 row-band pieces (big DMA descriptors),
small tensors in one blob DMA, rv transposed, emb normalized,
conv weights as [ci-dup128, expert, tap, cout].
"""
import sys

sys.path.insert(0, "/opt/trn_rl_repo")

import ml_dtypes
import numpy as np

import concourse.bass as bass
import concourse.mybir as mybir
from concourse.tile import TileContext

F32 = mybir.dt.float32
BF16 = mybir.dt.bfloat16
AF = mybir.ActivationFunctionType
ALU = mybir.AluOpType
AX = mybir.AxisListType

NCORES = 8
BLOC = 4           # samples per core
CIN = 64
COUT = 64
H = W = 58
HW = H * W         # 3364
HWP = HW + 4
OH = OW = 56
NB = 10            # experts
EDIM = 64
RSIZE = 512
HID = 128
NTAP = 9
CHUNK_ROWS = 8
NCHUNK = 7
NFREE = CHUNK_ROWS * W  # 464 <= 512 (one PSUM bank)
TAP_OFF = [dy * W + dx for dy in range(3) for dx in range(3)]
# x pieces: (first input row, n rows). chunk ch (input rows 8ch..8ch+9)
# reads piece ch//2; pieces overlap by 2 rows; each padded +4 elems
XPIECES = [(0, 18), (16, 18), (32, 18), (48, 10)]
XPN = [r * W + 4 for _, r in XPIECES]
CWP_GROUPS = [(0, 3), (3, 6), (6, 10)]
OGROUPS = [(0, 4), (4, 6), (6, 7)]
# blob A (routing-critical smalls), layout in fp32 columns
BA_RVT = (0, 8)        # bf16 [128, 4, 4]
BA_W2 = (8, 40)        # bf16 [128, 64]
BA_B1 = (40, 41)       # f32 [128, 1]
BA_B2 = (41, 42)       # f32 [64, 1]
BA_EMB = (42, 52)      # f32 [64, 10]
BA_CB = (52, 62)       # f32 [128, 10]
NBLOBA = 62
# blob B: W1 + identity
BB_W1 = (0, 256)       # bf16 [128, 4, 128]
BB_ID = (256, 320)     # bf16 [128, 128] identity
NBLOBB = 320


def fix_sync_waits(nc, cap=2):
    """This walrus build allows at most `cap` sem waits per instruction.
    Splice same-engine NoOps carrying the excess waits right before any
    over-subscribed instruction (waits happen earlier => same semantics)."""
    uid = [0]
    for f in nc.m.functions:
        for blk in f.blocks:
            insts = blk.instructions  # live list
            i = 0
            while i < len(insts):
                inst = insts[i]
                si = inst.sync_info
                waits = list(si.on_wait) if si and si.on_wait else []
                icap = 1
                if len(waits) <= icap:
                    i += 1
                    continue
                keep, excess = waits[-icap:], waits[:-icap]
                for k in range(0, len(excess), icap):
                    nop = mybir.InstNoOp(
                        name=f"{inst.name}-wsplit{uid[0]}", ins=[], outs=[]
                    )
                    uid[0] += 1
                    nop.engine = inst.engine
                    nop.sync_info = mybir.SyncInfo(
                        on_wait=excess[k : k + icap], on_update=[]
                    )
                    nc.register_instruction(nop, overwrite=True)
                    insts.insert(i, nop)
                    i += 1
                inst.sync_info = mybir.SyncInfo(
                    on_wait=keep,
                    on_update=list(si.on_update) if si and si.on_update else [],
                )
                i += 1
    return nc


def build():
    nc = bass.Bass()
    # partition layout p (all [128,...] tensors): p<64 -> ci=p, samples
    # {j0: b0, j1: b3}; p>=64 -> ci=p-64, samples {j0: b1, j1: b2}
    blobad = nc.dram_tensor("blobad", [128, NBLOBA], F32, kind="ExternalInput")
    blobbd = nc.dram_tensor("blobbd", [128, NBLOBB], F32, kind="ExternalInput")
    seld = nc.dram_tensor("seld", [BLOC, 4, 128], BF16, kind="ExternalInput")
    cwpd = nc.dram_tensor("cwpd", [128, NB, NTAP, COUT], BF16, kind="ExternalInput")
    xds = [
        nc.dram_tensor(f"x{k}d", [128, 2, n], BF16, kind="ExternalInput")
        for k, n in enumerate(XPN)
    ]
    out = nc.dram_tensor("out", [BLOC, COUT, OH, OW], BF16, kind="ExternalOutput")

    with TileContext(nc) as tc:
        with (
            tc.tile_pool(name="consts", bufs=1) as consts,
            tc.tile_pool(name="work", bufs=2) as work,
            tc.tile_pool(name="stage", bufs=2) as stage,
            tc.tile_pool(name="ps", bufs=2, space="PSUM") as pspool,
            tc.tile_pool(name="psw", bufs=1, space="PSUM") as pswpool,
            tc.tile_pool(name="psconv", bufs=2, space="PSUM") as psconv,
        ):
            # preload the ACT function table (1.3us) before any real work
            tbl = work.tile([1, 1], F32, tag="tbl")
            nc.vector.memset(tbl[:], 1.0)
            tbl2 = work.tile([1, 1], F32, tag="tbl2")
            nc.scalar.activation(out=tbl2[:], in_=tbl[:], func=AF.Exp)

            # ---------- input DMAs, one ring, priority order ----------
            bloba = consts.tile([128, NBLOBA], F32, tag="bloba")
            nc.sync.dma_start(out=bloba[:], in_=blobad[:])
            selsb = consts.tile([BLOC, 4, 128], BF16, tag="selsb")
            nc.sync.dma_start(out=selsb[:], in_=seld[:])
            blobb = consts.tile([128, NBLOBB], F32, tag="blobb")
            nc.sync.dma_start(out=blobb[:], in_=blobbd[:])
            cwp2 = consts.tile([128, NB, NTAP, COUT], BF16, tag="cwp2")
            for lo, hi in CWP_GROUPS:
                nc.sync.dma_start(out=cwp2[:, lo:hi], in_=cwpd[:, lo:hi])
            xsb = []
            for k, n in enumerate(XPN):
                t = consts.tile([128, 2, n], BF16, tag=f"xs{k}", name=f"xs{k}")
                nc.sync.dma_start(out=t[:], in_=xds[k][:])
                xsb.append(t)

            # views into the blobs
            rvt = bloba[:, BA_RVT[0] : BA_RVT[1]].bitcast(BF16).rearrange(
                "p (c b) -> p c b", b=BLOC
            )
            w2sb = bloba[:, BA_W2[0] : BA_W2[1]].bitcast(BF16)
            b1sb = bloba[:, BA_B1[0] : BA_B1[1]]
            b2sb = bloba[0:EDIM, BA_B2[0] : BA_B2[1]]
            embnt = bloba[0:EDIM, BA_EMB[0] : BA_EMB[1]]
            cb2 = bloba[:, BA_CB[0] : BA_CB[1]]
            w1sb = blobb[:, BB_W1[0] : BB_W1[1]].bitcast(BF16).rearrange(
                "p (c m) -> p c m", m=HID
            )
            identb = blobb[:, BB_ID[0] : BB_ID[1]].bitcast(BF16)
            ones64 = consts.tile([EDIM, 1], F32, tag="ones64")
            nc.vector.memset(ones64[:], 1.0)

            # ---------- routing MLP (f32 except the big W1 matmul) ----------
            h1 = pspool.tile([HID, BLOC], F32, tag="small")
            for c in range(4):
                nc.tensor.matmul(
                    h1[:], w1sb[:, c, :], rvt[:, c, :], start=(c == 0), stop=(c == 3)
                )
            h1r = work.tile([HID, BLOC], BF16, tag="h1r")
            nc.scalar.activation(
                out=h1r[:], in_=h1[:], func=AF.Relu, bias=b1sb, scale=1.0
            )
            rps = pspool.tile([EDIM, BLOC], F32, tag="small")
            nc.tensor.matmul(rps[:], w2sb, h1r[:], start=True, stop=True)
            rsb = work.tile([EDIM, BLOC], F32, tag="rsb")
            nc.scalar.activation(
                out=rsb[:], in_=rps[:], func=AF.Identity, bias=b2sb, scale=1.0
            )

            # 1/||r|| (emb pre-normalized on host)
            rsq = work.tile([EDIM, BLOC], F32, tag="rsq")
            nc.vector.tensor_mul(rsq[:], rsb[:], rsb[:])
            nsq = pspool.tile([BLOC, 1], F32, tag="small")
            nc.tensor.matmul(nsq[:], rsq[:], ones64[:], start=True, stop=True)
            rln = work.tile([BLOC, 1], F32, tag="rln")
            nc.scalar.activation(out=rln[:], in_=nsq[:], func=AF.Ln)
            rinv = work.tile([BLOC, 1], F32, tag="rinv")
            nc.scalar.activation(out=rinv[:], in_=rln[:], func=AF.Exp, scale=-0.5)

            # e = exp(cos) directly: cos in [-1,1], no max-subtraction needed
            simps = pspool.tile([BLOC, NB], F32, tag="small")
            nc.tensor.matmul(simps[:], rsb[:], embnt, start=True, stop=True)
            ex = work.tile([BLOC, NB], BF16, tag="ex")
            nc.scalar.activation(out=ex[:], in_=simps[:], func=AF.Exp, scale=rinv[:])

            # w_eff broadcast weights first (j0/j1 gate the diag chains)
            wfbc = []
            for j in range(2):
                ps = pspool.tile([128, NB], F32, tag="small")
                nc.tensor.matmul(ps[:], selsb[:, j, :], ex[:], start=True, stop=True)
                t = work.tile([128, NB], F32, tag=f"wfbc{j}")
                nc.scalar.copy(out=t[:], in_=ps[:])
                wfbc.append(t)

            # ---------- w_eff (bf16) ----------
            # weff[p, c, t, m]: c=0 -> (b0|b2), c=1 -> (b3|b1).
            # Taps 0-7 per expert on the PE: accumulate diag(wfbc_c[:,n])
            # @ cwp_n[taps 0-7] in PSUM (a diagonal stationary matrix
            # scales each partition row; one N=512 matmul per expert).
            # Tap 8 via fused scalar_tensor_tensor chains on DVE. Diags
            # built on DVE (c0) and ACT (c1). Warmup matmuls un-throttle
            # the PE clock (HAM) during this phase.
            weff = consts.tile([128, 2, NTAP, COUT], BF16, tag="weff")
            psw8a = pswpool.tile([128, 512], F32, tag="psw8a")
            psw8b = pswpool.tile([128, 512], F32, tag="psw8b")
            wrhs = w1sb.rearrange("p c m -> p (c m)")
            for _ in range(8):
                nc.tensor.matmul(psw8a[:], identb, wrhs, start=True, stop=True)

            def mkdiag(c, n, eng):
                dg = consts.tile(
                    [128, 128], BF16, tag=f"diag{c}{n}", name=f"diag{c}{n}"
                )
                if eng == "v":
                    nc.vector.tensor_scalar_mul(
                        out=dg[:], in0=identb, scalar1=wfbc[c][:, n : n + 1]
                    )
                else:
                    nc.scalar.activation(
                        out=dg[:], in_=identb, func=AF.Copy,
                        scale=wfbc[c][:, n : n + 1],
                    )
                return dg

            # c0 diags on DVE, c1 diags on ACT (first 6 before c0's copy)
            diags = {}
            for n in range(NB):
                diags[(0, n)] = mkdiag(0, n, "v")
            for n in range(6):
                diags[(1, n)] = mkdiag(1, n, "a")
            # tap-8 chains on DVE (after the c0 diags in DVE queue order)
            for c in range(2):
                nc.vector.tensor_scalar_mul(
                    out=weff[:, c, 8, :], in0=cwp2[:, 0, 8, :],
                    scalar1=wfbc[c][:, 0:1],
                )
                for n in range(1, NB):
                    nc.vector.scalar_tensor_tensor(
                        out=weff[:, c, 8, :], in0=cwp2[:, n, 8, :],
                        scalar=wfbc[c][:, n : n + 1], in1=weff[:, c, 8, :],
                        op0=ALU.mult, op1=ALU.add,
                    )
            # taps 0-7: c0 on PE, copy on ACT, then c1
            for n in range(NB):
                nc.tensor.matmul(
                    psw8a[:], diags[(0, n)][:], cwp2[:, n, 0:8, :],
                    start=(n == 0), stop=(n == NB - 1),
                )
            nc.scalar.activation(
                out=weff[:, 0, 0:8, :],
                in_=psw8a[:].rearrange("p (t m) -> p t m", m=COUT),
                func=AF.Copy,
            )
            for n in range(6, NB):
                diags[(1, n)] = mkdiag(1, n, "a")
            for n in range(NB):
                nc.tensor.matmul(
                    psw8b[:], diags[(1, n)][:], cwp2[:, n, 0:8, :],
                    start=(n == 0), stop=(n == NB - 1),
                )
            nc.scalar.activation(
                out=weff[:, 1, 0:8, :],
                in_=psw8b[:].rearrange("p (t m) -> p t m", m=COUT),
                func=AF.Copy,
            )

            # per-bank bias/scale broadcast (needed only at stage time)
            s = work.tile([BLOC, 1], F32, tag="s")
            nc.vector.tensor_reduce(s[:], ex[:], axis=AX.X, op=ALU.add)
            sinv = work.tile([BLOC, 1], F32, tag="sinv")
            nc.vector.reciprocal(sinv[:], s[:])
            exs = work.tile([BLOC, NB + 1], BF16, tag="exs")
            nc.vector.tensor_copy(out=exs[:, 0:NB], in_=ex[:])
            nc.vector.tensor_copy(out=exs[:, NB : NB + 1], in_=sinv[:])
            sinvbc = []
            for j in (2, 3):
                ps = pspool.tile([128, NB + 1], F32, tag="small")
                nc.tensor.matmul(ps[:], selsb[:, j, :], exs[:], start=True, stop=True)
                t = work.tile([128, NB + 1], F32, tag=f"wfbc{j}")
                nc.scalar.copy(out=t[:], in_=ps[:])
                wfbc.append(t)
                sinvbc.append(t[:, NB : NB + 1])
            beff = []
            for k, j in enumerate((2, 3)):
                junk = work.tile([128, NB], F32, tag="bjunk")
                acc = work.tile([128, 1], F32, tag=f"bacc{j}")
                nc.vector.scalar_tensor_tensor(
                    out=junk[:], in0=wfbc[j][:, 0:NB], scalar=1.0, in1=cb2,
                    op0=ALU.mult, op1=ALU.mult, accum_out=acc[:],
                )
                bt = work.tile([128, 1], F32, tag=f"beff{j}")
                nc.vector.tensor_mul(bt[:], acc[:], sinvbc[k])
                beff.append(bt)

            # ---------- conv: 7 chunks x 9 taps x 4 quadrant MMs ----------
            # quadrants: b0=(0,0) psA-low, b1=(64,64) psA-high,
            #            b2=(64,0) psB-low, b3=(0,64) psB-high
            gtile = {}
            g_of_chunk = {}
            for gi, (clo, chi) in enumerate(OGROUPS):
                for ch in range(clo, chi):
                    g_of_chunk[ch] = (gi, clo, chi)
            for ch in range(NCHUNK):
                h0 = ch * CHUNK_ROWS
                xs = xsb[ch // 2]
                base = (h0 - XPIECES[ch // 2][0]) * W
                psA = psconv.tile([128, NFREE], F32, tag="A")
                psB = psconv.tile([128, NFREE], F32, tag="B")
                for t in range(NTAP):
                    off = base + TAP_OFF[t]
                    st_, sp = (t == 0), (t == NTAP - 1)
                    nc.tensor.matmul(
                        psA[0:64, :], weff[0:64, 0, t, :],
                        xs[0:64, 0, off : off + NFREE], start=st_, stop=sp,
                    )
                    nc.tensor.matmul(
                        psA[64:128, :], weff[64:128, 1, t, :],
                        xs[64:128, 0, off : off + NFREE], start=st_, stop=sp,
                    )
                    nc.tensor.matmul(
                        psB[0:64, :], weff[64:128, 0, t, :],
                        xs[64:128, 1, off : off + NFREE], start=st_, stop=sp,
                    )
                    nc.tensor.matmul(
                        psB[64:128, :], weff[0:64, 1, t, :],
                        xs[0:64, 1, off : off + NFREE], start=st_, stop=sp,
                    )
                # stage: out = psum * sinv + bias, f32->bf16, trim to 56 cols
                # bankA on ACT, bankB on DVE so neither engine paces the PE
                gi, clo, chi = g_of_chunk[ch]
                rows = (chi - clo) * CHUNK_ROWS
                r = (ch - clo) * CHUNK_ROWS
                for bi, ps in ((0, psA), (1, psB)):
                    key = (bi, gi)
                    if key not in gtile:
                        stile = stage.tile(
                            [128, rows, OW], BF16, tag=f"st{bi}_{rows}",
                            name=f"st{bi}g{gi}",
                        )
                        gtile[key] = stile
                    stile = gtile[key]
                    psv = ps[:].rearrange("p (r w) -> p r w", w=W)[:, :, 0:OW]
                    if bi == 0:
                        nc.scalar.activation(
                            out=stile[:, r : r + CHUNK_ROWS, :], in_=psv,
                            func=AF.Identity, bias=beff[bi][:], scale=sinvbc[bi],
                        )
                    else:
                        nc.vector.tensor_scalar(
                            out=stile[:, r : r + CHUNK_ROWS, :], in0=psv,
                            scalar1=sinvbc[bi], scalar2=beff[bi][:],
                            op0=ALU.mult, op1=ALU.add,
                        )
                    if ch == chi - 1:  # group complete -> DMA out (2 rings,
                        # one DMA per bank covering both samples)
                        gh0 = clo * CHUNK_ROWS
                        eng = nc.gpsimd if bi == 0 else nc.sync
                        eng.dma_start(
                            out=out[2 * bi : 2 * bi + 2, :, gh0 : gh0 + rows, :]
                            .rearrange("b c r w -> (b c) r w"),
                            in_=stile[:],
                        )

    fix_sync_waits(nc)
    return nc


_NC = None


def _get_nc():
    global _NC
    if _NC is None:
        _NC = build()
    return _NC


def make_in_maps(inputs):
    bf16 = ml_dtypes.bfloat16

    def asf32(a):
        return np.ascontiguousarray(np.asarray(a, dtype=np.float32))

    def pack_bf16(a):
        # bf16 array -> f32-typed raw columns for the blob
        a = np.ascontiguousarray(a.astype(bf16))
        return a.reshape(a.shape[0], -1).view(np.float32)

    x = asf32(inputs["x"])
    rvec = asf32(inputs["routing_vector"])
    W1 = asf32(inputs["W1"])
    emb = asf32(inputs["emb"])
    conv_w = asf32(inputs["conv_w"])
    conv_b = asf32(inputs["conv_b"])

    embn = emb / (np.linalg.norm(emb, axis=-1, keepdims=True) + 1e-8)

    # conv_w[n, co, ci, ky, kx] -> [ci(dup 128), n, tap, co] bf16
    cwp = conv_w.transpose(2, 0, 3, 4, 1).reshape(CIN, NB, NTAP, COUT)
    cwpd = np.ascontiguousarray(np.concatenate([cwp, cwp], axis=0).astype(bf16))

    bloba_common = np.zeros((128, NBLOBA), np.float32)
    bloba_common[:, BA_W2[0] : BA_W2[1]] = pack_bf16(asf32(inputs["W2"]))
    bloba_common[:, BA_B1[0] : BA_B1[1]] = asf32(inputs["b1"]).reshape(HID, 1)
    bloba_common[0:EDIM, BA_B2[0] : BA_B2[1]] = asf32(inputs["b2"]).reshape(EDIM, 1)
    bloba_common[0:EDIM, BA_EMB[0] : BA_EMB[1]] = embn.T
    bloba_common[:, BA_CB[0] : BA_CB[1]] = np.tile(conv_b.T, (2, 1))
    blobb = np.zeros((128, NBLOBB), np.float32)
    # W1 [512,128] -> [128, 4, 128] bf16
    w1p = W1.reshape(4, 128, HID).transpose(1, 0, 2)
    blobb[:, BB_W1[0] : BB_W1[1]] = pack_bf16(w1p)
    blobb[:, BB_ID[0] : BB_ID[1]] = pack_bf16(np.eye(128, dtype=np.float32))
    selm = np.zeros((BLOC, 4, 128), np.float32)
    for j, (blo, bhi) in enumerate(((0, 2), (3, 1), (0, 1), (2, 3))):
        selm[blo, j, 0:64] = 1.0
        selm[bhi, j, 64:128] = 1.0
    selm = np.ascontiguousarray(selm.astype(bf16))

    in_maps = []
    for c in range(NCORES):
        xs = x[BLOC * c : BLOC * (c + 1)].reshape(BLOC, CIN, HW)
        # x[p, j, i]: p<64: (j0: b0, j1: b3); p>=64: (j0: b1, j1: b2)
        xa = np.zeros((128, 2, HWP), np.float32)
        xa[0:64, 0, 0:HW] = xs[0]
        xa[64:128, 0, 0:HW] = xs[1]
        xa[64:128, 1, 0:HW] = xs[2]
        xa[0:64, 1, 0:HW] = xs[3]
        bloba = bloba_common.copy()
        rvs = rvec[BLOC * c : BLOC * (c + 1)]                # [4, 512]
        rvt = rvs.T.reshape(4, 128, BLOC).transpose(1, 0, 2)  # [128, 4, 4]
        bloba[:, BA_RVT[0] : BA_RVT[1]] = pack_bf16(rvt)
        m = {"blobad": bloba, "blobbd": blobb, "seld": selm, "cwpd": cwpd}
        for k, (r0, nr) in enumerate(XPIECES):
            a = r0 * W
            m[f"x{k}d"] = np.ascontiguousarray(
                xa[:, :, a : a + XPN[k]].astype(bf16)
            )
        in_maps.append(m)
    return in_maps


def kernel(**inputs):
    from concourse.bass_utils import run_bass_kernel_spmd

    nc = _get_nc()
    in_maps = make_in_maps(inputs)
    res = run_bass_kernel_spmd(nc, in_maps, core_ids=list(range(NCORES)))
    return np.concatenate(
        [np.asarray(r["out"]).astype(np.float32) for r in res.results], axis=0
    )


# revision 41
# speedup vs baseline: 1.2555x; 1.2555x over previous
"""MoE routing layer on 8 Trainium2 NeuronCores (data-parallel over batch).

Per core (4 samples):
  routing MLP -> exp(cosine sim vs embeddings) -> unnormalized weights
  e[4,10]; w_eff[b] = sum_n e[b,n] * conv_w[n] (conv linear in weights);
  conv = 9 shifted bf16 matmuls over the flat 58-wide grid, all 4
  samples concurrent via 4-quadrant PE tiling; softmax normalization
  (1/sum e) and conv bias fold into the PSUM->SBUF staging op.

w_eff is built two ways concurrently: col-half 0 on the PE as a chain
of diagonal-stationary matmuls accumulating in PSUM (diag(wfbc_n) @
cwp_n), col-half 1 on DVE as tensor_scalar products + tensor_tensor
adds (ACT helps with late products). All heavy data is bf16; PSUM is
fp32. Host pre-packs: x in 4 row-band pieces (big DMA descriptors),
small tensors in one blob DMA, rv transposed, emb normalized,
conv weights as [ci-dup128, expert, tap, cout].
"""
import sys

sys.path.insert(0, "/opt/trn_rl_repo")

import ml_dtypes
import numpy as np

import concourse.bass as bass
import concourse.mybir as mybir
from concourse.tile import TileContext

F32 = mybir.dt.float32
BF16 = mybir.dt.bfloat16
AF = mybir.ActivationFunctionType
ALU = mybir.AluOpType
AX = mybir.AxisListType

NCORES = 8
BLOC = 4           # samples per core
CIN = 64
COUT = 64
H = W = 58
HW = H * W         # 3364
HWP = HW + 4
OH = OW = 56
NB = 10            # experts
EDIM = 64
RSIZE = 512
HID = 128
NTAP = 9
CHUNK_ROWS = 8
NCHUNK = 7
NFREE = CHUNK_ROWS * W  # 464 <= 512 (one PSUM bank)
TAP_OFF = [dy * W + dx for dy in range(3) for dx in range(3)]
# x pieces: (first input row, n rows). chunk ch (input rows 8ch..8ch+9)
# reads piece ch//2; pieces overlap by 2 rows; each padded +4 elems
XPIECES = [(0, 18), (16, 18), (32, 18), (48, 10)]
XPN = [r * W + 4 for _, r in XPIECES]
CWP_GROUPS = [(0, 5), (5, 10)]
OGROUPS = [(0, 4), (4, 6), (6, 7)]
# blob layout in fp32 columns: name -> (start, cols)
BL_RVT = (0, 8)        # bf16 [128, 4, 4]
BL_W1 = (8, 264)       # bf16 [128, 4, 128]
BL_W2 = (264, 296)     # bf16 [128, 64]
BL_B1 = (296, 297)     # f32 [128, 1]
BL_B2 = (297, 298)     # f32 [64, 1]
BL_EMB = (298, 308)    # f32 [64, 10]
BL_CB = (308, 318)     # f32 [128, 10]
BL_ID = (318, 382)     # bf16 [128, 128] identity
NBLOB = 382


def fix_sync_waits(nc, cap=2):
    """This walrus build allows at most `cap` sem waits per instruction.
    Splice same-engine NoOps carrying the excess waits right before any
    over-subscribed instruction (waits happen earlier => same semantics)."""
    uid = [0]
    for f in nc.m.functions:
        for blk in f.blocks:
            insts = blk.instructions  # live list
            i = 0
            while i < len(insts):
                inst = insts[i]
                si = inst.sync_info
                waits = list(si.on_wait) if si and si.on_wait else []
                icap = 1
                if len(waits) <= icap:
                    i += 1
                    continue
                keep, excess = waits[-icap:], waits[:-icap]
                for k in range(0, len(excess), icap):
                    nop = mybir.InstNoOp(
                        name=f"{inst.name}-wsplit{uid[0]}", ins=[], outs=[]
                    )
                    uid[0] += 1
                    nop.engine = inst.engine
                    nop.sync_info = mybir.SyncInfo(
                        on_wait=excess[k : k + icap], on_update=[]
                    )
                    nc.register_instruction(nop, overwrite=True)
                    insts.insert(i, nop)
                    i += 1
                inst.sync_info = mybir.SyncInfo(
                    on_wait=keep,
                    on_update=list(si.on_update) if si and si.on_update else [],
                )
                i += 1
    return nc


def build():
    nc = bass.Bass()
    # partition layout p (all [128,...] tensors): p<64 -> ci=p, samples
    # {j0: b0, j1: b3}; p>=64 -> ci=p-64, samples {j0: b1, j1: b2}
    blobd = nc.dram_tensor("blobd", [128, NBLOB], F32, kind="ExternalInput")
    seld = nc.dram_tensor("seld", [BLOC, 4, 128], BF16, kind="ExternalInput")
    cwpd = nc.dram_tensor("cwpd", [128, NB, NTAP, COUT], BF16, kind="ExternalInput")
    xds = [
        nc.dram_tensor(f"x{k}d", [128, 2, n], BF16, kind="ExternalInput")
        for k, n in enumerate(XPN)
    ]
    out = nc.dram_tensor("out", [BLOC, COUT, OH, OW], BF16, kind="ExternalOutput")

    with TileContext(nc) as tc:
        with (
            tc.tile_pool(name="consts", bufs=1) as consts,
            tc.tile_pool(name="work", bufs=2) as work,
            tc.tile_pool(name="stage", bufs=2) as stage,
            tc.tile_pool(name="ps", bufs=2, space="PSUM") as pspool,
            tc.tile_pool(name="psw", bufs=1, space="PSUM") as pswpool,
            tc.tile_pool(name="psconv", bufs=2, space="PSUM") as psconv,
        ):
            # preload the ACT function table (1.3us) before any real work
            tbl = work.tile([1, 1], F32, tag="tbl")
            nc.vector.memset(tbl[:], 1.0)
            tbl2 = work.tile([1, 1], F32, tag="tbl2")
            nc.scalar.activation(out=tbl2[:], in_=tbl[:], func=AF.Exp)

            # ---------- input DMAs, one ring, priority order ----------
            blob = consts.tile([128, NBLOB], F32, tag="blob")
            nc.sync.dma_start(out=blob[:], in_=blobd[:])
            selsb = consts.tile([BLOC, 4, 128], BF16, tag="selsb")
            nc.sync.dma_start(out=selsb[:], in_=seld[:])
            cwp2 = consts.tile([128, NB, NTAP, COUT], BF16, tag="cwp2")
            for lo, hi in CWP_GROUPS:
                nc.sync.dma_start(out=cwp2[:, lo:hi], in_=cwpd[:, lo:hi])
            xsb = []
            for k, n in enumerate(XPN):
                t = consts.tile([128, 2, n], BF16, tag=f"xs{k}", name=f"xs{k}")
                nc.sync.dma_start(out=t[:], in_=xds[k][:])
                xsb.append(t)

            # views into the blob
            rvt = blob[:, BL_RVT[0] : BL_RVT[1]].bitcast(BF16).rearrange(
                "p (c b) -> p c b", b=BLOC
            )
            w1sb = blob[:, BL_W1[0] : BL_W1[1]].bitcast(BF16).rearrange(
                "p (c m) -> p c m", m=HID
            )
            w2sb = blob[:, BL_W2[0] : BL_W2[1]].bitcast(BF16)
            b1sb = blob[:, BL_B1[0] : BL_B1[1]]
            b2sb = blob[0:EDIM, BL_B2[0] : BL_B2[1]]
            embnt = blob[0:EDIM, BL_EMB[0] : BL_EMB[1]]
            cb2 = blob[:, BL_CB[0] : BL_CB[1]]
            identb = blob[:, BL_ID[0] : BL_ID[1]].bitcast(BF16)
            ones64 = consts.tile([EDIM, 1], F32, tag="ones64")
            nc.vector.memset(ones64[:], 1.0)

            # ---------- routing MLP (f32 except the big W1 matmul) ----------
            h1 = pspool.tile([HID, BLOC], F32, tag="small")
            for c in range(4):
                nc.tensor.matmul(
                    h1[:], w1sb[:, c, :], rvt[:, c, :], start=(c == 0), stop=(c == 3)
                )
            h1r = work.tile([HID, BLOC], BF16, tag="h1r")
            nc.scalar.activation(
                out=h1r[:], in_=h1[:], func=AF.Relu, bias=b1sb, scale=1.0
            )
            rps = pspool.tile([EDIM, BLOC], F32, tag="small")
            nc.tensor.matmul(rps[:], w2sb, h1r[:], start=True, stop=True)
            rsb = work.tile([EDIM, BLOC], F32, tag="rsb")
            nc.scalar.activation(
                out=rsb[:], in_=rps[:], func=AF.Identity, bias=b2sb, scale=1.0
            )

            # 1/||r|| (emb pre-normalized on host)
            rsq = work.tile([EDIM, BLOC], F32, tag="rsq")
            nc.vector.tensor_mul(rsq[:], rsb[:], rsb[:])
            nsq = pspool.tile([BLOC, 1], F32, tag="small")
            nc.tensor.matmul(nsq[:], rsq[:], ones64[:], start=True, stop=True)
            rln = work.tile([BLOC, 1], F32, tag="rln")
            nc.scalar.activation(out=rln[:], in_=nsq[:], func=AF.Ln)
            rinv = work.tile([BLOC, 1], F32, tag="rinv")
            nc.scalar.activation(out=rinv[:], in_=rln[:], func=AF.Exp, scale=-0.5)

            # e = exp(cos) directly: cos in [-1,1], no max-subtraction needed
            simps = pspool.tile([BLOC, NB], F32, tag="small")
            nc.tensor.matmul(simps[:], rsb[:], embnt, start=True, stop=True)
            ex = work.tile([BLOC, NB], BF16, tag="ex")
            nc.scalar.activation(out=ex[:], in_=simps[:], func=AF.Exp, scale=rinv[:])

            # w_eff broadcast weights first (j0/j1 gate the diag chains)
            wfbc = []
            for j in range(2):
                ps = pspool.tile([128, NB], F32, tag="small")
                nc.tensor.matmul(ps[:], selsb[:, j, :], ex[:], start=True, stop=True)
                t = work.tile([128, NB], F32, tag=f"wfbc{j}")
                nc.scalar.copy(out=t[:], in_=ps[:])
                wfbc.append(t)

            # ---------- w_eff (bf16) ----------
            # weff[p, c, t, m]: c=0 -> (b0|b2), c=1 -> (b3|b1).
            # Taps 0-7 per expert on the PE: accumulate diag(wfbc_c[:,n])
            # @ cwp_n[taps 0-7] in PSUM (a diagonal stationary matrix
            # scales each partition row; one N=512 matmul per expert).
            # Tap 8 via fused scalar_tensor_tensor chains on DVE. Diags
            # built on DVE (c0) and ACT (c1). Warmup matmuls un-throttle
            # the PE clock (HAM) during this phase.
            weff = consts.tile([128, 2, NTAP, COUT], BF16, tag="weff")
            psw8a = pswpool.tile([128, 512], F32, tag="psw8a")
            psw8b = pswpool.tile([128, 512], F32, tag="psw8b")
            wrhs = w1sb.rearrange("p c m -> p (c m)")
            for _ in range(8):
                nc.tensor.matmul(psw8a[:], identb, wrhs, start=True, stop=True)
            diags = {}
            for c in range(2):
                for n in range(NB):
                    dg = consts.tile(
                        [128, 128], BF16, tag=f"diag{c}{n}", name=f"diag{c}{n}"
                    )
                    if c == 0:
                        nc.vector.tensor_scalar_mul(
                            out=dg[:], in0=identb, scalar1=wfbc[c][:, n : n + 1]
                        )
                    else:
                        nc.scalar.activation(
                            out=dg[:], in_=identb, func=AF.Copy,
                            scale=wfbc[c][:, n : n + 1],
                        )
                    diags[(c, n)] = dg
            # tap-8 chains on DVE
            for c in range(2):
                nc.vector.tensor_scalar_mul(
                    out=weff[:, c, 8, :], in0=cwp2[:, 0, 8, :],
                    scalar1=wfbc[c][:, 0:1],
                )
                for n in range(1, NB):
                    nc.vector.scalar_tensor_tensor(
                        out=weff[:, c, 8, :], in0=cwp2[:, n, 8, :],
                        scalar=wfbc[c][:, n : n + 1], in1=weff[:, c, 8, :],
                        op0=ALU.mult, op1=ALU.add,
                    )
            # taps 0-7 on PE + staging copies on ACT
            for c, psw8 in ((0, psw8a), (1, psw8b)):
                for n in range(NB):
                    nc.tensor.matmul(
                        psw8[:], diags[(c, n)][:], cwp2[:, n, 0:8, :],
                        start=(n == 0), stop=(n == NB - 1),
                    )
                nc.scalar.activation(
                    out=weff[:, c, 0:8, :],
                    in_=psw8[:].rearrange("p (t m) -> p t m", m=COUT),
                    func=AF.Copy,
                )

            # per-bank bias/scale broadcast (needed only at stage time)
            s = work.tile([BLOC, 1], F32, tag="s")
            nc.vector.tensor_reduce(s[:], ex[:], axis=AX.X, op=ALU.add)
            sinv = work.tile([BLOC, 1], F32, tag="sinv")
            nc.vector.reciprocal(sinv[:], s[:])
            exs = work.tile([BLOC, NB + 1], BF16, tag="exs")
            nc.vector.tensor_copy(out=exs[:, 0:NB], in_=ex[:])
            nc.vector.tensor_copy(out=exs[:, NB : NB + 1], in_=sinv[:])
            sinvbc = []
            for j in (2, 3):
                ps = pspool.tile([128, NB + 1], F32, tag="small")
                nc.tensor.matmul(ps[:], selsb[:, j, :], exs[:], start=True, stop=True)
                t = work.tile([128, NB + 1], F32, tag=f"wfbc{j}")
                nc.scalar.copy(out=t[:], in_=ps[:])
                wfbc.append(t)
                sinvbc.append(t[:, NB : NB + 1])
            beff = []
            for k, j in enumerate((2, 3)):
                junk = work.tile([128, NB], F32, tag="bjunk")
                acc = work.tile([128, 1], F32, tag=f"bacc{j}")
                nc.vector.scalar_tensor_tensor(
                    out=junk[:], in0=wfbc[j][:, 0:NB], scalar=1.0, in1=cb2,
                    op0=ALU.mult, op1=ALU.mult, accum_out=acc[:],
                )
                bt = work.tile([128, 1], F32, tag=f"beff{j}")
                nc.vector.tensor_mul(bt[:], acc[:], sinvbc[k])
                beff.append(bt)

            # ---------- conv: 7 chunks x 9 taps x 4 quadrant MMs ----------
            # quadrants: b0=(0,0) psA-low, b1=(64,64) psA-high,
            #            b2=(64,0) psB-low, b3=(0,64) psB-high
            gtile = {}
            g_of_chunk = {}
            for gi, (clo, chi) in enumerate(OGROUPS):
                for ch in range(clo, chi):
                    g_of_chunk[ch] = (gi, clo, chi)
            for ch in range(NCHUNK):
                h0 = ch * CHUNK_ROWS
                xs = xsb[ch // 2]
                base = (h0 - XPIECES[ch // 2][0]) * W
                psA = psconv.tile([128, NFREE], F32, tag="A")
                psB = psconv.tile([128, NFREE], F32, tag="B")
                for t in range(NTAP):
                    off = base + TAP_OFF[t]
                    st_, sp = (t == 0), (t == NTAP - 1)
                    nc.tensor.matmul(
                        psA[0:64, :], weff[0:64, 0, t, :],
                        xs[0:64, 0, off : off + NFREE], start=st_, stop=sp,
                    )
                    nc.tensor.matmul(
                        psA[64:128, :], weff[64:128, 1, t, :],
                        xs[64:128, 0, off : off + NFREE], start=st_, stop=sp,
                    )
                    nc.tensor.matmul(
                        psB[0:64, :], weff[64:128, 0, t, :],
                        xs[64:128, 1, off : off + NFREE], start=st_, stop=sp,
                    )
                    nc.tensor.matmul(
                        psB[64:128, :], weff[0:64, 1, t, :],
                        xs[0:64, 1, off : off + NFREE], start=st_, stop=sp,
                    )
                # stage: out = psum * sinv + bias, f32->bf16, trim to 56 cols
                # bankA on ACT, bankB on DVE so neither engine paces the PE
                gi, clo, chi = g_of_chunk[ch]
                rows = (chi - clo) * CHUNK_ROWS
                r = (ch - clo) * CHUNK_ROWS
                for bi, ps in ((0, psA), (1, psB)):
                    key = (bi, gi)
                    if key not in gtile:
                        stile = stage.tile(
                            [128, rows, OW], BF16, tag=f"st{bi}_{rows}",
                            name=f"st{bi}g{gi}",
                        )
                        gtile[key] = stile
                    stile = gtile[key]
                    psv = ps[:].rearrange("p (r w) -> p r w", w=W)[:, :, 0:OW]
                    if bi == 0:
                        nc.scalar.activation(
                            out=stile[:, r : r + CHUNK_ROWS, :], in_=psv,
                            func=AF.Identity, bias=beff[bi][:], scale=sinvbc[bi],
                        )
                    else:
                        nc.vector.tensor_scalar(
                            out=stile[:, r : r + CHUNK_ROWS, :], in0=psv,
                            scalar1=sinvbc[bi], scalar2=beff[bi][:],
                            op0=ALU.mult, op1=ALU.add,
                        )
                    if ch == chi - 1:  # group complete -> DMA out (2 rings,
                        # one DMA per bank covering both samples)
                        gh0 = clo * CHUNK_ROWS
                        eng = nc.gpsimd if bi == 0 else nc.sync
                        eng.dma_start(
                            out=out[2 * bi : 2 * bi + 2, :, gh0 : gh0 + rows, :]
                            .rearrange("b c r w -> (b c) r w"),
                            in_=stile[:],
                        )

    fix_sync_waits(nc)
    return nc


_NC = None


def _get_nc():
    global _NC
    if _NC is None:
        _NC = build()
    return _NC


def make_in_maps(inputs):
    bf16 = ml_dtypes.bfloat16

    def asf32(a):
        return np.ascontiguousarray(np.asarray(a, dtype=np.float32))

    def pack_bf16(a):
        # bf16 array -> f32-typed raw columns for the blob
        a = np.ascontiguousarray(a.astype(bf16))
        return a.reshape(a.shape[0], -1).view(np.float32)

    x = asf32(inputs["x"])
    rvec = asf32(inputs["routing_vector"])
    W1 = asf32(inputs["W1"])
    emb = asf32(inputs["emb"])
    conv_w = asf32(inputs["conv_w"])
    conv_b = asf32(inputs["conv_b"])

    embn = emb / (np.linalg.norm(emb, axis=-1, keepdims=True) + 1e-8)

    # conv_w[n, co, ci, ky, kx] -> [ci(dup 128), n, tap, co] bf16
    cwp = conv_w.transpose(2, 0, 3, 4, 1).reshape(CIN, NB, NTAP, COUT)
    cwpd = np.ascontiguousarray(np.concatenate([cwp, cwp], axis=0).astype(bf16))

    blob_common = np.zeros((128, NBLOB), np.float32)
    # W1 [512,128] -> [128, 4, 128] bf16
    w1p = W1.reshape(4, 128, HID).transpose(1, 0, 2)
    blob_common[:, BL_W1[0] : BL_W1[1]] = pack_bf16(w1p)
    blob_common[:, BL_W2[0] : BL_W2[1]] = pack_bf16(asf32(inputs["W2"]))
    blob_common[:, BL_B1[0] : BL_B1[1]] = asf32(inputs["b1"]).reshape(HID, 1)
    blob_common[0:EDIM, BL_B2[0] : BL_B2[1]] = asf32(inputs["b2"]).reshape(EDIM, 1)
    blob_common[0:EDIM, BL_EMB[0] : BL_EMB[1]] = embn.T
    blob_common[:, BL_CB[0] : BL_CB[1]] = np.tile(conv_b.T, (2, 1))
    blob_common[:, BL_ID[0] : BL_ID[1]] = pack_bf16(np.eye(128, dtype=np.float32))
    selm = np.zeros((BLOC, 4, 128), np.float32)
    for j, (blo, bhi) in enumerate(((0, 2), (3, 1), (0, 1), (2, 3))):
        selm[blo, j, 0:64] = 1.0
        selm[bhi, j, 64:128] = 1.0
    selm = np.ascontiguousarray(selm.astype(bf16))

    in_maps = []
    for c in range(NCORES):
        xs = x[BLOC * c : BLOC * (c + 1)].reshape(BLOC, CIN, HW)
        # x[p, j, i]: p<64: (j0: b0, j1: b3); p>=64: (j0: b1, j1: b2)
        xa = np.zeros((128, 2, HWP), np.float32)
        xa[0:64, 0, 0:HW] = xs[0]
        xa[64:128, 0, 0:HW] = xs[1]
        xa[64:128, 1, 0:HW] = xs[2]
        xa[0:64, 1, 0:HW] = xs[3]
        blob = blob_common.copy()
        rvs = rvec[BLOC * c : BLOC * (c + 1)]                # [4, 512]
        rvt = rvs.T.reshape(4, 128, BLOC).transpose(1, 0, 2)  # [128, 4, 4]
        blob[:, BL_RVT[0] : BL_RVT[1]] = pack_bf16(rvt)
        m = {"blobd": blob, "seld": selm, "cwpd": cwpd}
        for k, (r0, nr) in enumerate(XPIECES):
            a = r0 * W
            m[f"x{k}d"] = np.ascontiguousarray(
                xa[:, :, a : a + XPN[k]].astype(bf16)
            )
        in_maps.append(m)
    return in_maps


def kernel(**inputs):
    from concourse.bass_utils import run_bass_kernel_spmd

    nc = _get_nc()
    in_maps = make_in_maps(inputs)
    res = run_bass_kernel_spmd(nc, in_maps, core_ids=list(range(NCORES)))
    return np.concatenate(
        [np.asarray(r["out"]).astype(np.float32) for r in res.results], axis=0
    )
